# revision 10
# baseline (speedup 1.0000x reference)
"""Fused GPT-attention block (QKV proj -> causal attention -> out proj) on 8
Trainium2 NeuronCores.

Sharding: core c handles batch b = c//2 and head-group hg = c%2 (8 of 16
heads).  QKV/proj weights are column/row-sliced per head-group on the host;
attention is embarrassingly parallel over (batch, head).  The per-core output
projection produces a partial sum over its 512 hidden channels; the host adds
the two partials per batch plus the projection bias.

Device kernel layout choices (per core):
  - x arrives pre-transposed (host) as xT [1024, 2048] so the contraction dim
    (channels) sits on SBUF partitions for every matmul.
  - q,k are produced transposed ("qT/kT" = [head-d, t]) packed two heads per
    128-partition tile; v is produced in natural [t, head-d] layout with a
    ones-column per head (for softmax denominators via the PE).
  - scores are computed transposed: sT[k, q] = kT.T @ qT, masked causally by
    construction (only lower tiles computed; diagonal tiles get an additive
    host-supplied [-1e9] mask), exp'd on the scalar engine (no max-subtraction:
    scores are O(1) so fp32 exp is safe), then av^T accumulates in PSUM over
    k-tiles with the ones-column producing the softmax denominator row.
  - matmuls run in float32r (full fp32 data, fast PE streaming mode).
"""

import sys

sys.path.insert(0, "/opt/trn_rl_repo")

import numpy as np

B, T, NX, NH = 4, 2048, 1024, 16
HD = 64
NCORES = 8
HPC = 8  # heads per core
PAIRS = 4  # head pairs per core
CK = 8  # channel chunks of 128
TT = 16  # t tiles of 128

_CACHE = {}


def _build_nc(reps=1):
    import concourse.mybir as mybir
    import concourse.tile as tile
    from concourse import bacc
    from contextlib import ExitStack

    dt = mybir.dt
    F32 = dt.float32
    F32R = dt.float32r
    AF = mybir.ActivationFunctionType

    def r(ap):  # matmul-feeding tiles are declared float32r end-to-end
        return ap

    nc = bacc.Bacc("TRN2", target_bir_lowering=False, debug=False,
                   num_devices=NCORES)

    xT_d = nc.dram_tensor("xT", [NX, T], F32R, kind="ExternalInput")
    wqk_d = nc.dram_tensor("wqk", [NX, 1024], F32R, kind="ExternalInput")
    wv_d = nc.dram_tensor("wv", [NX, 512], F32R, kind="ExternalInput")
    wp_d = nc.dram_tensor("wp", [512, NX], F32R, kind="ExternalInput")
    bqk_d = nc.dram_tensor("bqk", [128, 8], F32, kind="ExternalInput")
    vb_d = nc.dram_tensor("vb", [128, 512], F32, kind="ExternalInput")
    dm_d = nc.dram_tensor("dmask", [128, 128], F32, kind="ExternalInput")
    on64_d = nc.dram_tensor("ones64", [1, 64], F32R, kind="ExternalInput")
    on8_d = nc.dram_tensor("ones8", [128, 8], F32R, kind="ExternalInput")
    kt_out = nc.dram_tensor("kt_out", [512, T], F32R, kind="ExternalOutput")
    v_out = nc.dram_tensor("v_out", [T, 512], F32R, kind="ExternalOutput")
    p_out = nc.dram_tensor("p_out", [T, NX], F32, kind="ExternalOutput")

    lp = nc.allow_low_precision(
        reason="float32r tiles: fp32 data in PE fast-stream rounding format")
    with lp, tile.TileContext(nc) as tc, ExitStack() as top:
        cpool = top.enter_context(tc.tile_pool(name="const", bufs=1))
        dmask = cpool.tile([128, 128], F32)
        nc.sync.dma_start(out=dmask[:], in_=dm_d.ap())
        vb = cpool.tile([128, 512], F32)
        nc.sync.dma_start(out=vb[:], in_=vb_d.ap())
        bqk = cpool.tile([128, 8], F32)
        nc.sync.dma_start(out=bqk[:], in_=bqk_d.ap())
        ones = cpool.tile([1, 64], F32R)
        nc.sync.dma_start(out=ones[:], in_=on64_d.ap())
        ones8 = cpool.tile([128, 8], F32R)
        nc.sync.dma_start(out=ones8[:], in_=on8_d.ap())

        for rep in range(reps):
            with ExitStack() as body:
                # persistent across phases of one rep
                qkpool = body.enter_context(
                    tc.tile_pool(name=f"qk{rep}", bufs=1))
                # 0-3: qT pairs, 4-7: kT pairs  ([2*HD, T] packed)
                qkT = [qkpool.tile([128, T], F32R, tag=f"qkT{i}", name=f"qkT{rep}_{i}")
                       for i in range(8)]
                vpool = body.enter_context(
                    tc.tile_pool(name=f"vx{rep}", bufs=1))
                # v_ext: per t-tile 8 groups of (64 v-cols + 1 ones-col)
                vext = vpool.tile([128, TT * 520], F32R)
                # ones-columns (softmax denominator) per head group, per t-tile
                for gi in range(TT):
                    nc.sync.dma_start(
                        out=vext[:, gi * 520:(gi + 1) * 520]
                        .rearrange("p (h c) -> p h c", h=8)[:, :, 64:65],
                        in_=ones8[:].rearrange("p (h c) -> p h c", h=8))

                # ---------------- phase A: qkv projection -----------------
                with ExitStack() as ph:
                    wpool = ph.enter_context(
                        tc.tile_pool(name=f"w{rep}", bufs=1))
                    wqk_sb = [wpool.tile([128, 1024], F32R, tag=f"wqk{j}", name=f"wqk{rep}_{j}")
                              for j in range(CK)]
                    wv_sb = [wpool.tile([128, 512], F32R, tag=f"wv{j}", name=f"wv{rep}_{j}")
                            for j in range(CK)]
                    for j in range(CK):
                        nc.sync.dma_start(out=wqk_sb[j][:],
                                          in_=wqk_d.ap()[j * 128:(j + 1) * 128, :])
                        nc.sync.dma_start(out=wv_sb[j][:],
                                          in_=wv_d.ap()[j * 128:(j + 1) * 128, :])
                    xpool = ph.enter_context(
                        tc.tile_pool(name=f"x{rep}", bufs=1))
                    pspool = ph.enter_context(
                        tc.tile_pool(name=f"psA{rep}", bufs=4, space="PSUM"))

                    for half in range(2):  # t halves of 1024
                        xT_sb = [xpool.tile([128, 1024], F32R, tag=f"xt{j}", name=f"xt{rep}_{j}")
                                 for j in range(CK)]
                        for j in range(CK):
                            nc.sync.dma_start(
                                out=xT_sb[j][:],
                                in_=xT_d.ap()[j * 128:(j + 1) * 128,
                                              half * 1024:(half + 1) * 1024])
                        # qT / kT : stationary W cols (2 heads), moving xT
                        for qk in range(2):
                            for p in range(PAIRS):
                                for tc_ in range(2):  # 512-chunks in half
                                    ps = pspool.tile([128, 512], F32, tag="psA", name=f"psA{rep}")
                                    for j in range(CK):
                                        nc.tensor.matmul(
                                            ps[:],
                                            r(wqk_sb[j][:, qk * 512 + p * 128:
                                                        qk * 512 + (p + 1) * 128]),
                                            r(xT_sb[j][:, tc_ * 512:(tc_ + 1) * 512]),
                                            start=(j == 0), stop=(j == CK - 1))
                                    nc.scalar.activation(
                                        qkT[qk * 4 + p][:, half * 1024 + tc_ * 512:
                                                        half * 1024 + (tc_ + 1) * 512],
                                        ps[:], AF.Identity,
                                        bias=bqk[:, qk * 4 + p:qk * 4 + p + 1])
                        # v natural: stationary xT tile, moving Wv
                        for it in range(8):  # t-tiles within half
                            gi = half * 8 + it
                            ps = pspool.tile([128, 512], F32, tag="psA", name=f"psAv{rep}")
                            for j in range(CK):
                                nc.tensor.matmul(
                                    ps[:],
                                    r(xT_sb[j][:, it * 128:(it + 1) * 128]),
                                    r(wv_sb[j][:]),
                                    start=(j == 0), stop=(j == CK - 1))
                            # copy + bias into v_ext (strided per-head 65-col
                            # groups; ones columns stay at 1.0)
                            vdst = vext[:, gi * 520:(gi + 1) * 520] \
                                .rearrange("p (h c) -> p h c", h=8)[:, :, 0:64]
                            nc.vector.tensor_add(
                                vdst,
                                ps[:].rearrange("p (h c) -> p h c", h=8),
                                vb[:].rearrange("p (h c) -> p h c", h=8))
                            nc.sync.dma_start(
                                out=v_out.ap()[gi * 128:(gi + 1) * 128, :],
                                in_=vdst)

                # kT -> present output
                for p in range(PAIRS):
                    nc.sync.dma_start(out=kt_out.ap()[p * 128:(p + 1) * 128, :],
                                      in_=qkT[4 + p][:])

                # ---------------- phase C: attention ----------------------
                avpool = body.enter_context(
                    tc.tile_pool(name=f"avT{rep}", bufs=1))
                avT = [avpool.tile([128, T], F32R, tag=f"avT{p}", name=f"avT{rep}_{p}")
                       for p in range(PAIRS)]
                wppool = body.enter_context(
                    tc.tile_pool(name=f"wp{rep}", bufs=1))
                wp_sb = [wppool.tile([128, 1024], F32R, tag=f"wp{p}", name=f"wp{rep}_{p}")
                         for p in range(PAIRS)]
                for p in range(PAIRS):
                    nc.sync.dma_start(out=wp_sb[p][:],
                                      in_=wp_d.ap()[p * 128:(p + 1) * 128, :])

                with ExitStack() as ph:
                    stpool = ph.enter_context(
                        tc.tile_pool(name=f"st{rep}", bufs=2, space="PSUM"))
                    avps_pool = ph.enter_context(
                        tc.tile_pool(name=f"avps{rep}", bufs=1, space="PSUM"))
                    espool = ph.enter_context(
                        tc.tile_pool(name=f"es{rep}", bufs=3))
                    rpool = ph.enter_context(
                        tc.tile_pool(name=f"r{rep}", bufs=2))
                    rbpool = ph.enter_context(
                        tc.tile_pool(name=f"rb{rep}", bufs=2))

                    for h in range(HPC):
                        p2, off = h // 2, (h % 2) * 64
                        av_ps = avps_pool.tile([128, T], F32, tag="av", name=f"avps{rep}")
                        for kt in range(TT):
                            qs = kt * 128
                            for win in (0, 1024):
                                if win + 1024 <= qs:
                                    continue
                                lo = max(qs - win, 0)
                                sT = stpool.tile([128, 1024], F32, tag="sT", name=f"sT{rep}")
                                # chunks aligned to 512 within the window
                                chunks = []
                                if lo < 512:
                                    chunks.append((lo, 512))
                                    chunks.append((512, 1024))
                                else:
                                    chunks.append((lo, 1024))
                                for (c0, c1) in chunks:
                                    nc.tensor.matmul(
                                        sT[:, c0:c1],
                                        r(qkT[4 + p2][off:off + 64,
                                                      kt * 128:(kt + 1) * 128]),
                                        r(qkT[p2][off:off + 64,
                                                  win + c0:win + c1]),
                                        start=True, stop=True)
                                if win <= qs < win + 1024:
                                    nc.vector.tensor_add(
                                        sT[:, lo:lo + 128],
                                        sT[:, lo:lo + 128], dmask[:])
                                es = espool.tile([128, 1024], F32R, tag="es", name=f"es{rep}")
                                nc.scalar.activation(
                                    es[:, lo:1024], sT[:, lo:1024],
                                    AF.Exp, scale=0.125)
                                for (c0, c1) in chunks:
                                    last_kt = (win + c1 - 1) // 128
                                    nc.tensor.matmul(
                                        av_ps[0:65, win + c0:win + c1],
                                        r(vext[:, kt * 520 + 65 * h:
                                               kt * 520 + 65 * h + 65]),
                                        r(es[:, c0:c1]),
                                        start=(kt == 0),
                                        stop=(kt == min(last_kt, TT - 1)))
                        # normalize: avT = av / denom-row
                        rrow = rpool.tile([1, T], F32R, tag="r", name=f"rrow{rep}")
                        nc.vector.reciprocal(rrow[:], av_ps[64:65, :])
                        for w2 in range(2):
                            rb = stpool.tile([128, 1024], F32, tag="sT", name=f"rb{rep}")
                            for cc in range(2):
                                nc.tensor.matmul(
                                    rb[0:64, cc * 512:(cc + 1) * 512],
                                    r(ones[:]),
                                    r(rrow[:, w2 * 1024 + cc * 512:
                                           w2 * 1024 + (cc + 1) * 512]),
                                    start=True, stop=True)
                            # DVE can read only one PSUM operand: stage the
                            # broadcast recip through SBUF first.
                            rbs = rbpool.tile([64, 1024], F32, tag="rbs",
                                              name=f"rbs{rep}")
                            nc.vector.tensor_copy(rbs[:], rb[0:64, :])
                            nc.vector.tensor_mul(
                                avT[p2][off:off + 64,
                                        w2 * 1024:(w2 + 1) * 1024],
                                av_ps[0:64, w2 * 1024:(w2 + 1) * 1024],
                                rbs[:])

                # ---------------- phase D: output projection --------------
                with ExitStack() as ph:
                    psD = ph.enter_context(
                        tc.tile_pool(name=f"psD{rep}", bufs=4, space="PSUM"))
                    opool = ph.enter_context(
                        tc.tile_pool(name=f"o{rep}", bufs=4))
                    for i in range(TT):
                        for nh in range(2):
                            ps = psD.tile([128, 512], F32, tag="psD", name=f"psD{rep}")
                            for p in range(PAIRS):
                                nc.tensor.matmul(
                                    ps[:],
                                    r(avT[p][:, i * 128:(i + 1) * 128]),
                                    r(wp_sb[p][:, nh * 512:(nh + 1) * 512]),
                                    start=(p == 0), stop=(p == PAIRS - 1))
                            ot = opool.tile([128, 512], F32, tag="o", name=f"ot{rep}")
                            nc.scalar.activation(ot[:], ps[:], AF.Copy)
                            nc.sync.dma_start(
                                out=p_out.ap()[i * 128:(i + 1) * 128,
                                               nh * 512:(nh + 1) * 512],
                                in_=ot[:])

    nc.compile()
    return nc


def _get_nc(reps=1):
    if reps not in _CACHE:
        _CACHE[reps] = _build_nc(reps)
    return _CACHE[reps]


def _prep_core_inputs(x, c_attn_w, c_attn_b, c_proj_w):
    dmask = np.where(np.arange(128)[:, None] > np.arange(128)[None, :],
                     np.float32(-1e9), np.float32(0.0))
    ins = []
    for c in range(NCORES):
        b, hg = c // 2, c % 2
        s = hg * 512
        wqk = np.concatenate([c_attn_w[:, s:s + 512],
                              c_attn_w[:, 1024 + s:1024 + s + 512]], axis=1)
        bqk = np.empty((128, 8), np.float32)
        for qk in range(2):
            for p in range(PAIRS):
                bqk[:, qk * 4 + p] = c_attn_b[qk * 1024 + s + p * 128:
                                              qk * 1024 + s + (p + 1) * 128]
        vb = np.broadcast_to(c_attn_b[2048 + s:2048 + s + 512],
                             (128, 512)).copy()
        ins.append({
            "ones64": np.ones((1, 64), np.float32),
            "ones8": np.ones((128, 8), np.float32),
            "xT": np.ascontiguousarray(x[b].T),
            "wqk": np.ascontiguousarray(wqk),
            "wv": np.ascontiguousarray(c_attn_w[:, 2048 + s:2048 + s + 512]),
            "wp": np.ascontiguousarray(c_proj_w[s:s + 512, :]),
            "bqk": bqk,
            "vb": vb,
            "dmask": dmask,
        })
    return ins


def kernel(x, mask_self_attention, c_attn_w, c_attn_b, c_proj_w, c_proj_b):
    from concourse.bass_utils import run_bass_kernel_spmd

    x = np.asarray(x, np.float32)
    c_attn_w = np.asarray(c_attn_w, np.float32)
    c_attn_b = np.asarray(c_attn_b, np.float32)
    c_proj_w = np.asarray(c_proj_w, np.float32)
    c_proj_b = np.asarray(c_proj_b, np.float32)

    nc = _get_nc(1)
    ins = _prep_core_inputs(x, c_attn_w, c_attn_b, c_proj_w)
    res = run_bass_kernel_spmd(nc, ins, core_ids=list(range(NCORES))).results

    a = np.empty((B, T, NX), np.float32)
    present = np.empty((2, B, NH, T, HD), np.float32)
    for c in range(NCORES):
        b, hg = c // 2, c % 2
        hs = hg * 8
        kt = res[c]["kt_out"]  # [512, T] rows = h_local*64 + d
        present[0, b, hs:hs + 8] = kt.reshape(8, 64, T).transpose(0, 2, 1)
        v = res[c]["v_out"]    # [T, 512] cols = h_local*64 + d
        present[1, b, hs:hs + 8] = v.reshape(T, 8, 64).transpose(1, 0, 2)
    for b in range(B):
        a[b] = res[2 * b]["p_out"] + res[2 * b + 1]["p_out"] + c_proj_b
    return a, present


# revision 15
# speedup vs baseline: 6.6721x; 6.6721x over previous
"""Fused GPT-attention block (QKV proj -> causal attention -> out proj) on 8
Trainium2 NeuronCores.

Sharding: core c handles batch b = c//2 and head-group hg = c%2 (8 of 16
heads).  QKV/proj weights are column/row-sliced per head-group on the host;
attention is embarrassingly parallel over (batch, head).  The per-core output
projection produces a partial sum over its 512 hidden channels; the host adds
the two partials per batch plus the projection bias.

Device kernel layout choices (per core):
  - x arrives pre-transposed (host) as xT [1024, 2048] so the contraction dim
    (channels) sits on SBUF partitions for every matmul; it is streamed in
    512-column chunks (double buffered) so DMA overlaps the QKV matmuls.
  - q,k are produced transposed ("qT/kT" = [head-d, t]) packed two heads per
    128-partition tile; v is produced in natural [t, head-d] layout with a
    ones-column per head (softmax denominators fall out of the PE).
  - scores are computed transposed: sT[k, q] = kT.T @ qT, causally masked by
    construction (only lower k-tiles computed; diagonal tiles get an additive
    host-supplied [-1e9] mask), exp'd on the scalar engine (no max-subtraction:
    scores are O(1) in this problem so fp32 exp cannot overflow), then av^T
    accumulates in PSUM over k-tiles, processed in two independent 1024-wide
    q-windows so two heads can pipeline in PSUM.
  - matmuls run in float32r: fp32 data in the PE's fast-streaming format
    (~1 cycle/row like bf16, ~16-bit mantissa precision).
"""

import sys

sys.path.insert(0, "/opt/trn_rl_repo")

import numpy as np

B, T, NX, NH = 4, 2048, 1024, 16
HD = 64
NCORES = 8
HPC = 8  # heads per core
PAIRS = 4  # head pairs per core
CK = 8  # channel chunks of 128
TT = 16  # t tiles of 128

_CACHE = {}


def _build_nc(reps=1, phases=("A", "C", "D"), loop=0):
    import concourse.mybir as mybir
    import concourse.tile as tile
    from concourse import bacc
    from contextlib import ExitStack

    dt = mybir.dt
    F32 = dt.float32
    F32R = dt.float32r
    AF = mybir.ActivationFunctionType

    nc = bacc.Bacc("TRN2", target_bir_lowering=False, debug=False,
                   num_devices=NCORES)

    xT_d = nc.dram_tensor("xT", [NX, T], F32R, kind="ExternalInput")
    wqk_d = nc.dram_tensor("wqk", [NX, 1024], F32R, kind="ExternalInput")
    wv_d = nc.dram_tensor("wv", [NX, 512], F32R, kind="ExternalInput")
    wp_d = nc.dram_tensor("wp", [512, NX], F32R, kind="ExternalInput")
    bqk_d = nc.dram_tensor("bqk", [128, 8], F32, kind="ExternalInput")
    vb_d = nc.dram_tensor("vb", [128, 512], F32, kind="ExternalInput")
    dm_d = nc.dram_tensor("dmask", [128, 128], F32, kind="ExternalInput")
    on64_d = nc.dram_tensor("ones64", [1, 64], F32R, kind="ExternalInput")
    kt_out = nc.dram_tensor("kt_out", [512, T], F32R, kind="ExternalOutput")
    v_out = nc.dram_tensor("v_out", [T, 512], F32, kind="ExternalOutput")
    p_out = nc.dram_tensor("p_out", [T, NX], F32, kind="ExternalOutput")

    dphases = [p for p in phases if p.startswith("D")]
    dmode = dphases[0] if dphases else None

    lp = nc.allow_low_precision(
        reason="float32r tiles: fp32 data in PE fast-stream rounding format")
    with lp, tile.TileContext(nc) as tc, ExitStack() as top:
        cpool = top.enter_context(tc.tile_pool(name="const", bufs=1))
        dmask = cpool.tile([128, 128], F32)
        nc.sync.dma_start(out=dmask[:], in_=dm_d.ap())
        vb = cpool.tile([128, 512], F32)
        nc.sync.dma_start(out=vb[:], in_=vb_d.ap())
        bqk = cpool.tile([128, 8], F32)
        nc.sync.dma_start(out=bqk[:], in_=bqk_d.ap())
        ones = cpool.tile([1, 64], F32R)
        nc.sync.dma_start(out=ones[:], in_=on64_d.ap())

        loop_cm = tc.For_i(0, loop, 1) if loop else None
        if loop_cm is not None:
            loop_cm.__enter__()

        for rep in range(reps):
            with ExitStack() as body:
                qkpool = body.enter_context(
                    tc.tile_pool(name=f"qk{rep}", bufs=1))
                # 0-3: qT pairs, 4-7: kT pairs  ([2*HD, T] packed)
                qkT = [qkpool.tile([128, T], F32R, tag=f"qkT{i}",
                                   name=f"qkT{rep}_{i}") for i in range(8)]
                vpool = body.enter_context(
                    tc.tile_pool(name=f"vx{rep}", bufs=1))
                # v_ext: per t-tile 8 groups of (64 v-cols + 1 ones-col)
                vext = vpool.tile([128, TT * 520], F32R)

                # ------------- phase A: qkv projection ----------------
                with ExitStack() as ph:
                  if "A" in phases:
                    wpool = ph.enter_context(
                        tc.tile_pool(name=f"w{rep}", bufs=1))
                    wqk_sb = [wpool.tile([128, 1024], F32R, tag=f"wqk{j}",
                                         name=f"wqk{rep}_{j}")
                              for j in range(CK)]
                    wv_sb = [wpool.tile([128, 512], F32R, tag=f"wv{j}",
                                        name=f"wv{rep}_{j}")
                            for j in range(CK)]
                    for j in range(CK):
                        nc.sync.dma_start(out=wqk_sb[j][:],
                                          in_=wqk_d.ap()[j * 128:(j + 1) * 128, :])
                        nc.sync.dma_start(out=wv_sb[j][:],
                                          in_=wv_d.ap()[j * 128:(j + 1) * 128, :])
                    xcpool = ph.enter_context(
                        tc.tile_pool(name=f"x{rep}", bufs=2))
                    vspool = ph.enter_context(
                        tc.tile_pool(name=f"vs{rep}", bufs=3))
                    pspool = ph.enter_context(
                        tc.tile_pool(name=f"psA{rep}", bufs=4, space="PSUM"))

                    for tck in range(4):  # t chunks of 512
                        xc = [xcpool.tile([128, 512], F32R, tag=f"xc{j}",
                                          name=f"xc{rep}_{j}")
                              for j in range(CK)]
                        for j in range(CK):
                            nc.sync.dma_start(
                                out=xc[j][:],
                                in_=xT_d.ap()[j * 128:(j + 1) * 128,
                                              tck * 512:(tck + 1) * 512])
                        # qT / kT : stationary W cols (2 heads), moving xT
                        for qk in range(2):
                            for p in range(PAIRS):
                                ps = pspool.tile([128, 512], F32, tag="psA",
                                                 name=f"psA{rep}")
                                for j in range(CK):
                                    nc.tensor.matmul(
                                        ps[:],
                                        wqk_sb[j][:, qk * 512 + p * 128:
                                                  qk * 512 + (p + 1) * 128],
                                        xc[j][:],
                                        start=(j == 0), stop=(j == CK - 1))
                                nc.vector.tensor_scalar_add(
                                    qkT[qk * 4 + p][:, tck * 512:(tck + 1) * 512],
                                    ps[:],
                                    bqk[:, qk * 4 + p:qk * 4 + p + 1])
                        # v natural: stationary xT tile, moving Wv
                        for it in range(4):  # t-tiles in this chunk
                            gi = tck * 4 + it
                            ps = pspool.tile([128, 512], F32, tag="psA",
                                             name=f"psAv{rep}")
                            for j in range(CK):
                                nc.tensor.matmul(
                                    ps[:],
                                    xc[j][:, it * 128:(it + 1) * 128],
                                    wv_sb[j][:],
                                    start=(j == 0), stop=(j == CK - 1))
                            vs = vspool.tile([128, 512], F32, tag="vs",
                                             name=f"vs{rep}")
                            nc.vector.tensor_add(vs[:], ps[:], vb[:])
                            nc.sync.dma_start(
                                out=v_out.ap()[gi * 128:(gi + 1) * 128, :],
                                in_=vs[:])
                            # strided copy into v_ext 65-col head groups
                            nc.scalar.activation(
                                vext[:, gi * 520:(gi + 1) * 520]
                                .rearrange("p (h c) -> p h c", h=8)[:, :, 0:64],
                                vs[:].rearrange("p (h c) -> p h c", h=8),
                                AF.Copy)
                            # ones columns for the softmax denominator:
                            # vs*0 + 1 via tensor_scalar(mult, add)
                            nc.vector.tensor_scalar(
                                vext[:, gi * 520:(gi + 1) * 520]
                                .rearrange("p (h c) -> p h c", h=8)[:, :, 64:65],
                                vs[:].rearrange("p (h c) -> p h c", h=8)[:, :, 0:1],
                                0.0, 1.0,
                                op0=mybir.AluOpType.mult,
                                op1=mybir.AluOpType.add)

                    # kT -> present output
                    for p in range(PAIRS):
                        nc.sync.dma_start(
                            out=kt_out.ap()[p * 128:(p + 1) * 128, :],
                            in_=qkT[4 + p][:])

                # ------------- phase C: attention ---------------------
                avpool = body.enter_context(
                    tc.tile_pool(name=f"avT{rep}", bufs=1))
                avT = [avpool.tile([128, T], F32R, tag=f"avT{p}",
                                   name=f"avT{rep}_{p}") for p in range(PAIRS)]
                wppool = body.enter_context(
                    tc.tile_pool(name=f"wp{rep}", bufs=1))
                wp_sb = [wppool.tile([128, 1024], F32R, tag=f"wp{p}",
                                     name=f"wp{rep}_{p}") for p in range(PAIRS)]
                for p in range(PAIRS):
                    nc.sync.dma_start(out=wp_sb[p][:],
                                      in_=wp_d.ap()[p * 128:(p + 1) * 128, :])

                with ExitStack() as ph:
                  if "C" in phases:
                    stpool = ph.enter_context(
                        tc.tile_pool(name=f"st{rep}", bufs=2, space="PSUM"))
                    avps_pool = ph.enter_context(
                        tc.tile_pool(name=f"avps{rep}", bufs=2, space="PSUM"))
                    espool = ph.enter_context(
                        tc.tile_pool(name=f"es{rep}", bufs=3))
                    rpool = ph.enter_context(
                        tc.tile_pool(name=f"r{rep}", bufs=2))
                    rbpool = ph.enter_context(
                        tc.tile_pool(name=f"rb{rep}", bufs=2))

                    for h in range(HPC):
                        p2, off = h // 2, (h % 2) * 64
                        for win in (0, 1024):
                            ktn = win // 128 + 8  # k-tiles feeding this window
                            av = avps_pool.tile([65, 1024], F32, tag="av",
                                                name=f"avps{rep}")

                            def emit_av(kt, es, chunks, av=av, win=win,
                                        ktn=ktn, h=h):
                                for (c0, c1) in chunks:
                                    last_kt = min(ktn - 1,
                                                  (win + c1 - 1) // 128)
                                    nc.tensor.matmul(
                                        av[0:65, c0:c1],
                                        vext[:, kt * 520 + 65 * h:
                                             kt * 520 + 65 * h + 65],
                                        es[:, c0:c1],
                                        start=(kt == 0),
                                        stop=(kt == last_kt))

                            pending = None  # software-pipeline: av lags scores
                            for kt in range(ktn):
                                qs = kt * 128
                                lo = max(qs - win, 0)
                                sT = stpool.tile([128, 1024], F32, tag="sT",
                                                 name=f"sT{rep}")
                                chunks = ([(lo, 512), (512, 1024)]
                                          if lo < 512 else [(lo, 1024)])
                                for (c0, c1) in chunks:
                                    nc.tensor.matmul(
                                        sT[:, c0:c1],
                                        qkT[4 + p2][off:off + 64,
                                                    kt * 128:(kt + 1) * 128],
                                        qkT[p2][off:off + 64,
                                                win + c0:win + c1],
                                        start=True, stop=True)
                                if win <= qs:  # diagonal tile: causal mask
                                    nc.vector.tensor_add(
                                        sT[:, lo:lo + 128],
                                        sT[:, lo:lo + 128], dmask[:])
                                es = espool.tile([128, 1024], F32R, tag="es",
                                                 name=f"es{rep}")
                                nc.scalar.activation(
                                    es[:, lo:1024], sT[:, lo:1024],
                                    AF.Exp, scale=0.125)
                                if pending is not None:
                                    emit_av(*pending)
                                pending = (kt, es, chunks)
                            emit_av(*pending)
                            # normalize this window: avT = av / denom-row
                            rrow = rpool.tile([1, 1024], F32R, tag="r",
                                              name=f"rrow{rep}")
                            nc.vector.reciprocal(rrow[:], av[64:65, :])
                            rb = stpool.tile([128, 1024], F32, tag="sT",
                                             name=f"rb{rep}")
                            for cc in range(2):
                                nc.tensor.matmul(
                                    rb[0:64, cc * 512:(cc + 1) * 512],
                                    ones[:],
                                    rrow[:, cc * 512:(cc + 1) * 512],
                                    start=True, stop=True)
                            rbs = rbpool.tile([64, 1024], F32, tag="rbs",
                                              name=f"rbs{rep}")
                            nc.vector.tensor_copy(rbs[:], rb[0:64, :])
                            nc.vector.tensor_mul(
                                avT[p2][off:off + 64, win:win + 1024],
                                av[0:64, :], rbs[:])

                # ------------- phase D: output projection -------------
                with ExitStack() as ph:
                  if dmode:
                    psD = ph.enter_context(
                        tc.tile_pool(name=f"psD{rep}", bufs=4, space="PSUM"))
                    opool = ph.enter_context(
                        tc.tile_pool(name=f"o{rep}", bufs=3))
                    for i in range(TT):
                        ot = opool.tile([128, 1024], F32, tag="o",
                                        name=f"ot{rep}")
                        for nh in range(2):
                            if dmode != "Do":
                                ps = psD.tile([128, 512], F32, tag="psD",
                                              name=f"psD{rep}")
                                for p in range(PAIRS):
                                    nc.tensor.matmul(
                                        ps[:],
                                        avT[p][:, i * 128:(i + 1) * 128],
                                        wp_sb[p][:, nh * 512:(nh + 1) * 512],
                                        start=(p == 0), stop=(p == PAIRS - 1))
                                nc.scalar.activation(
                                    ot[:, nh * 512:(nh + 1) * 512], ps[:],
                                    AF.Copy)
                        if dmode != "Dn":
                            nc.sync.dma_start(
                                out=p_out.ap()[i * 128:(i + 1) * 128, :],
                                in_=ot[:])

        if loop_cm is not None:
            loop_cm.__exit__(None, None, None)
    nc.compile()
    return nc


def _get_nc(reps=1, phases=("A", "C", "D"), loop=0):
    key = (reps, phases, loop)
    if key not in _CACHE:
        _CACHE[key] = _build_nc(reps, phases, loop)
    return _CACHE[key]


def _prep_core_inputs(x, c_attn_w, c_attn_b, c_proj_w):
    dmask = np.where(np.arange(128)[:, None] > np.arange(128)[None, :],
                     np.float32(-1e9), np.float32(0.0))
    ins = []
    for c in range(NCORES):
        b, hg = c // 2, c % 2
        s = hg * 512
        wqk = np.concatenate([c_attn_w[:, s:s + 512],
                              c_attn_w[:, 1024 + s:1024 + s + 512]], axis=1)
        bqk = np.empty((128, 8), np.float32)
        for qk in range(2):
            for p in range(PAIRS):
                bqk[:, qk * 4 + p] = c_attn_b[qk * 1024 + s + p * 128:
                                              qk * 1024 + s + (p + 1) * 128]
        vb = np.broadcast_to(c_attn_b[2048 + s:2048 + s + 512],
                             (128, 512)).copy()
        ins.append({
            "ones64": np.ones((1, 64), np.float32),
            "xT": np.ascontiguousarray(x[b].T),
            "wqk": np.ascontiguousarray(wqk),
            "wv": np.ascontiguousarray(c_attn_w[:, 2048 + s:2048 + s + 512]),
            "wp": np.ascontiguousarray(c_proj_w[s:s + 512, :]),
            "bqk": bqk,
            "vb": vb,
            "dmask": dmask,
        })
    return ins


def kernel(x, mask_self_attention, c_attn_w, c_attn_b, c_proj_w, c_proj_b):
    from concourse.bass_utils import run_bass_kernel_spmd

    x = np.asarray(x, np.float32)
    c_attn_w = np.asarray(c_attn_w, np.float32)
    c_attn_b = np.asarray(c_attn_b, np.float32)
    c_proj_w = np.asarray(c_proj_w, np.float32)
    c_proj_b = np.asarray(c_proj_b, np.float32)

    nc = _get_nc(1)
    ins = _prep_core_inputs(x, c_attn_w, c_attn_b, c_proj_w)
    res = run_bass_kernel_spmd(nc, ins, core_ids=list(range(NCORES))).results

    a = np.empty((B, T, NX), np.float32)
    present = np.empty((2, B, NH, T, HD), np.float32)
    for c in range(NCORES):
        b, hg = c // 2, c % 2
        hs = hg * 8
        kt = res[c]["kt_out"]  # [512, T] rows = h_local*64 + d
        present[0, b, hs:hs + 8] = kt.reshape(8, 64, T).transpose(0, 2, 1)
        v = res[c]["v_out"]    # [T, 512] cols = h_local*64 + d
        present[1, b, hs:hs + 8] = v.reshape(T, 8, 64).transpose(1, 0, 2)
    for b in range(B):
        a[b] = res[2 * b]["p_out"] + res[2 * b + 1]["p_out"] + c_proj_b
    return a, present
